# revision 5
# baseline (speedup 1.0000x reference)
"""Trainium2 Bass kernel for nn_BINLayer (binarized dense layer).

Computes out = sign(x) @ sign(W) + sign(bias) with sign(v >= 0) = +1 else -1
(forward value of the straight-through-estimator reference).

Strategy:
  - Data-parallel shard x over batch rows: 8 cores x 1024 rows each.
    W and bias are replicated; each core computes its full [1024, 4096]
    output slice, results are concatenated on the host.
  - The host ships x transposed ([D, B_shard]) so the contraction dim lands
    on SBUF partitions, and cast to bf16 (sign-exact: bf16 has the full f32
    exponent range, so sign(bf16(v)) == sign(v) for every input value).
  - On device: sign is computed on the Scalar engine (ACT Sign activation),
    emitting +-1 directly in fp8e4. The matmul runs on the Tensor engine in
    fp8 DoubleRow mode (2 fp8 weights per PE cell, contraction 256/matmul)
    with fp32 PSUM accumulation. Since all operands are exactly +-1 and row
    sums are integers <= 4097, the result is bit-exact vs float64.
  - Bias (sign-converted on device) is added during PSUM->SBUF eviction on
    the Vector engine, fused with the copy.
"""

import os
from contextlib import ExitStack

import numpy as np
import ml_dtypes

import concourse.bass as bass
from concourse import mybir
from concourse.bass_utils import run_bass_kernel_spmd

P = 128
D = 4096
B = 8192
N_CORES = 8
B_SHARD = B // N_CORES  # 1024
NFREE = 512  # psum free dim (one bank of fp32)

F32 = mybir.dt.float32
BF16 = mybir.dt.bfloat16
FP8 = mybir.dt.float8e4

SIGN = mybir.ActivationFunctionType.Sign

# Stash of the most recent BassKernelResults (exec_time_ns etc) for test.py.
LAST_RESULTS = None


def build_nc(d=D, b_shard=B_SHARD, nfree=NFREE, use_fp8=True):
    """Build the per-core Bass program (raw bass: explicit engine streams and
    semaphores — the toolchain here rejects DMAs carrying >1 sync wait, so all
    waits are sequencer instructions). Every core runs this same program on
    its own batch shard.

    Engine assignment:
      SP (sync)    input DMAs (bias, xT tiles, W tiles)
      ACT (scalar) all sign conversions (bf16 -> +-1 in fp8)
      PE (tensor)  matmuls (fp8 DoubleRow, fp32 PSUM accumulate)
      DVE (vector) PSUM->SBUF eviction fused with bias add
      POOL(gpsimd) output DMAs
    """
    KT = d // P        # contraction tiles of 128
    MT = b_shard // P  # output row tiles of 128
    NT = d // nfree    # output col blocks of nfree
    KK = KT // 2       # DoubleRow pairs
    NB_X = min(4, KT)  # x staging ring slots
    NB_W = min(8, KT)  # W staging ring slots
    NB_O = 8           # out staging ring slots
    NGRP = NT * MT     # psum accumulation groups
    cdt = FP8 if use_fp8 else BF16

    nc = bass.Bass()
    xT = nc.declare_dram_parameter("xT", [d, b_shard], BF16, isOutput=False)
    W = nc.declare_dram_parameter("W", [d, d], BF16, isOutput=False)
    bias_b = nc.declare_dram_parameter("bias_b", [P, d], F32, isOutput=False)
    out = nc.declare_dram_parameter("out", [b_shard, d], F32, isOutput=True)

    with ExitStack() as ctx:
        ent = ctx.enter_context
        xsb = ent(nc.sbuf_tensor("xsb", [P, NB_X, b_shard], BF16))
        bx = ent(nc.sbuf_tensor("bx", [P, KT, b_shard], cdt))
        wsb = ent(nc.sbuf_tensor("wsb", [P, NB_W, nfree], BF16))
        wb = ent(nc.sbuf_tensor("wb", [P, 2, KT, nfree], cdt))
        bsb = ent(nc.sbuf_tensor("bsb", [P, d], F32))
        osb = ent(nc.sbuf_tensor("osb", [P, NB_O, nfree], F32))
        pst = ent(nc.psum_tensor("pst", [P, 8, nfree], F32))

        s_bd = ent(nc.semaphore("s_bd"))   # bias dma done (+16)
        s_bs = ent(nc.semaphore("s_bs"))   # bias signed (+1)
        s_xs = ent(nc.semaphore("s_xs"))   # x signs done (+1 each)
        s_ws = ent(nc.semaphore("s_ws"))   # W signs done (+1 each)
        s_mm = ent(nc.semaphore("s_mm"))   # psum groups done (+1 each)
        s_ev = ent(nc.semaphore("s_ev"))   # evict+bias adds done (+1 each)
        # Per-slot DMA-completion sems: DMA completions across a shared sem
        # are unordered, so "sem >= 16*(k+1)" would not mean "DMA k landed".
        # One sem per ring slot with at most one DMA in flight per slot makes
        # the thresholds sound.
        s_xd = [ent(nc.semaphore(f"s_xd{i}")) for i in range(NB_X)]
        s_wd = [ent(nc.semaphore(f"s_wd{i}")) for i in range(NB_W)]
        s_od = [ent(nc.semaphore(f"s_od{i}")) for i in range(NB_O)]
        all_sems = [s_bd, s_bs, s_xs, s_ws, s_mm, s_ev, *s_xd, *s_wd, *s_od]

        def wslice(n):
            return slice(n * nfree, (n + 1) * nfree)

        with nc.Block() as block:

            @block.sync
            def _(sync):
                sync.dma_start(out=bsb[:, :], in_=bias_b[:, :]).then_inc(s_bd, 16)
                # prologue: interleave x tiles with W block-0 tiles
                for kt in range(KT):
                    if kt >= NB_X:
                        sync.wait_ge(s_xs, kt - NB_X + 1)
                    sync.dma_start(
                        out=xsb[:, kt % NB_X, :], in_=xT[kt * P:(kt + 1) * P, :]
                    ).then_inc(s_xd[kt % NB_X], 16)
                    if kt >= NB_W:
                        sync.wait_ge(s_ws, kt - NB_W + 1)
                    sync.dma_start(
                        out=wsb[:, kt % NB_W, :], in_=W[kt * P:(kt + 1) * P, wslice(0)]
                    ).then_inc(s_wd[kt % NB_W], 16)
                for n in range(1, NT):
                    for kt in range(KT):
                        j = n * KT + kt
                        sync.wait_ge(s_ws, j - NB_W + 1)
                        sync.dma_start(
                            out=wsb[:, j % NB_W, :],
                            in_=W[kt * P:(kt + 1) * P, wslice(n)],
                        ).then_inc(s_wd[j % NB_W], 16)

            @block.scalar
            def _(scalar):
                scalar.wait_ge(s_bd, 16)
                scalar.activation(bsb[:, :], bsb[:, :], SIGN).then_inc(s_bs, 1)
                for kt in range(KT):
                    scalar.wait_ge(s_xd[kt % NB_X], 16 * (kt // NB_X + 1))
                    scalar.activation(
                        bx[:, kt, :], xsb[:, kt % NB_X, :], SIGN
                    ).then_inc(s_xs, 1)
                    scalar.wait_ge(s_wd[kt % NB_W], 16 * (kt // NB_W + 1))
                    scalar.activation(
                        wb[:, 0, kt, :], wsb[:, kt % NB_W, :], SIGN
                    ).then_inc(s_ws, 1)
                for n in range(1, NT):
                    for kt in range(KT):
                        j = n * KT + kt
                        if kt == 0 and n >= 2:
                            # wb slot n%2 is free once PE drained block n-2
                            scalar.wait_ge(s_mm, MT * (n - 1))
                        scalar.wait_ge(s_wd[j % NB_W], 16 * (j // NB_W + 1))
                        scalar.activation(
                            wb[:, n % 2, kt, :], wsb[:, j % NB_W, :], SIGN
                        ).then_inc(s_ws, 1)

            @block.tensor
            def _(tensor):
                for n in range(NT):
                    for m in range(MT):
                        g = n * MT + m
                        if n > 0 and m == 0:
                            tensor.wait_ge(s_ws, KT * (n + 1))
                        if g >= 8:
                            tensor.wait_ge(s_ev, g - 7)
                        for kk in range(KK):
                            if n == 0:
                                # block 0 is k-pipelined: start each matmul as
                                # soon as its x/W k-pair is sign-converted
                                tensor.wait_ge(s_xs, 2 * kk + 2)
                                tensor.wait_ge(s_ws, 2 * kk + 2)
                            mm = tensor.matmul(
                                pst[:, g % 8, :],
                                bx[:, 2 * kk:2 * kk + 2, m * P:(m + 1) * P],
                                wb[:, n % 2, 2 * kk:2 * kk + 2, :],
                                start=(kk == 0),
                                stop=(kk == KK - 1),
                                perf_mode=mybir.MatmulPerfMode.DoubleRow,
                            )
                        mm.then_inc(s_mm, 1)

            @block.vector
            def _(vector):
                vector.wait_ge(s_bs, 1)
                for g in range(NGRP):
                    n = g // MT
                    vector.wait_ge(s_mm, g + 1)
                    if g >= NB_O:
                        vector.wait_ge(s_od[g % NB_O], 16 * (g // NB_O))
                    vector.tensor_add(
                        osb[:, g % NB_O, :], pst[:, g % 8, :], bsb[:, wslice(n)]
                    ).then_inc(s_ev, 1)

            @block.gpsimd
            def _(gpsimd):
                for g in range(NGRP):
                    n, m = g // MT, g % MT
                    gpsimd.wait_ge(s_ev, g + 1)
                    gpsimd.dma_start(
                        out=out[m * P:(m + 1) * P, wslice(n)],
                        in_=osb[:, g % NB_O, :],
                    ).then_inc(s_od[g % NB_O], 16)
                # drain own DMAs before the end-of-block barrier
                for i in range(NB_O):
                    n_dmas = len(range(i, NGRP, NB_O))
                    gpsimd.wait_ge(s_od[i], 16 * n_dmas)

        # Block exit emitted drain + all-engine barrier: every stream is done.
        # Zero the semaphores so a re-execution of the loaded NEFF starts clean.
        for s in all_sems:
            nc.sync.sem_clear(s)

    return nc


def _prep_inputs(x, W, bias):
    """Host-side shard/layout prep: transpose x, cast to bf16 (sign-exact),
    replicate bias across the 128 partitions."""
    xT = np.ascontiguousarray(np.asarray(x).astype(ml_dtypes.bfloat16).T)
    Wb = np.ascontiguousarray(np.asarray(W).astype(ml_dtypes.bfloat16))
    bias_b = np.ascontiguousarray(
        np.broadcast_to(np.asarray(bias).astype(np.float32)[None, :], (P, D))
    )
    in_maps = []
    for c in range(N_CORES):
        in_maps.append(
            {
                "xT": np.ascontiguousarray(xT[:, c * B_SHARD:(c + 1) * B_SHARD]),
                "W": Wb,
                "bias_b": bias_b,
            }
        )
    return in_maps


def kernel(x, W, bias):
    global LAST_RESULTS
    in_maps = _prep_inputs(x, W, bias)
    nc = build_nc()
    res = run_bass_kernel_spmd(
        nc,
        in_maps,
        core_ids=list(range(N_CORES)),
        trace=bool(int(os.environ.get("KBASS_TRACE", "0"))),
    )
    LAST_RESULTS = res
    out = np.concatenate([r["out"] for r in res.results], axis=0)
    return np.ascontiguousarray(out.astype(np.float32))


# revision 10
# speedup vs baseline: 1.0643x; 1.0643x over previous
"""Trainium2 Bass kernel for nn_BINLayer (binarized dense layer).

Computes out = sign(x) @ sign(W) + sign(bias) with sign(v >= 0) = +1 else -1
(forward value of the straight-through-estimator reference).

Strategy:
  - Data-parallel shard x over batch rows: 8 cores x 1024 rows each.
    W and bias are replicated; each core computes its full [1024, 4096]
    output slice, results are concatenated on the host.
  - The host ships x transposed ([D, B_shard]) so the contraction dim lands
    on SBUF partitions, and cast to bf16 (sign-exact: bf16 has the full f32
    exponent range, so sign(bf16(v)) == sign(v) for every input value).
  - On device: sign is computed on the Scalar engine (ACT Sign activation),
    emitting +-1 directly in fp8e4. The matmul runs on the Tensor engine in
    fp8 DoubleRow mode (2 fp8 weights per PE cell, contraction 256/matmul)
    with fp32 PSUM accumulation. Since all operands are exactly +-1 and row
    sums are integers <= 4097, the result is bit-exact vs float64.
  - Bias (sign-converted on device) is added during PSUM->SBUF eviction on
    the Vector engine, fused with the copy.
"""

import os
from contextlib import ExitStack

import numpy as np
import ml_dtypes

import concourse.bass as bass
from concourse import mybir
from concourse.bass_utils import run_bass_kernel_spmd

P = 128
D = 4096
B = 8192
N_CORES = 8
B_SHARD = B // N_CORES  # 1024
NFREE = 512  # psum free dim (one bank of fp32)

F32 = mybir.dt.float32
BF16 = mybir.dt.bfloat16
FP8 = mybir.dt.float8e4

SIGN = mybir.ActivationFunctionType.Sign

# Stash of the most recent BassKernelResults (exec_time_ns etc) for test.py.
LAST_RESULTS = None


def build_nc(d=D, b_shard=B_SHARD, nfree=NFREE, use_fp8=True):
    """Build the per-core Bass program (raw bass: explicit engine streams and
    semaphores — the toolchain here rejects DMAs carrying >1 sync wait, so all
    waits are sequencer instructions). Every core runs this same program on
    its own batch shard.

    Engine assignment:
      SP (sync)    input DMAs (bias, xT tiles, W tiles)
      ACT (scalar) all sign conversions (bf16 -> +-1 in fp8)
      PE (tensor)  matmuls (fp8 DoubleRow, fp32 PSUM accumulate)
      DVE (vector) PSUM->SBUF eviction fused with bias add
      POOL(gpsimd) output DMAs
    """
    KT = d // P        # contraction tiles of 128
    MT = b_shard // P  # output row tiles of 128
    NT = d // nfree    # output col blocks of nfree
    KK = KT // 2       # DoubleRow pairs
    NB_X = min(4, KT)  # x staging ring slots
    NB_W = min(8, KT)  # W staging ring slots
    NB_O = 8           # out staging ring slots
    NGRP = NT * MT     # psum accumulation groups
    cdt = FP8 if use_fp8 else BF16

    nc = bass.Bass()
    xT = nc.declare_dram_parameter("xT", [d, b_shard], BF16, isOutput=False)
    W = nc.declare_dram_parameter("W", [d, d], BF16, isOutput=False)
    bias_b = nc.declare_dram_parameter("bias_b", [P, d], F32, isOutput=False)
    out = nc.declare_dram_parameter("out", [b_shard, d], F32, isOutput=True)

    with ExitStack() as ctx:
        ent = ctx.enter_context
        xsb = ent(nc.sbuf_tensor("xsb", [P, NB_X, b_shard], BF16))
        bx = ent(nc.sbuf_tensor("bx", [P, KT, b_shard], cdt))
        wsb = ent(nc.sbuf_tensor("wsb", [P, NB_W, nfree], BF16))
        wb = ent(nc.sbuf_tensor("wb", [P, 2, KT, nfree], cdt))
        bsb = ent(nc.sbuf_tensor("bsb", [P, d], F32))
        osb = ent(nc.sbuf_tensor("osb", [P, NB_O, nfree], F32))
        pst = [ent(nc.psum_tensor(f"pst{b}", [P, nfree], F32)) for b in range(8)]

        s_bd = ent(nc.semaphore("s_bd"))   # bias dma done (+16)
        s_bs = ent(nc.semaphore("s_bs"))   # bias signed (+1)
        s_xs = ent(nc.semaphore("s_xs"))   # x signs done (+1 each, ACT)
        # W signs are split across two engines so the prologue isn't
        # serialized on ACT: DVE signs blocks 0..1 (via a uint8 bit trick),
        # ACT signs blocks 2+. Separate sems keep each count single-producer
        # so "sem >= k" == "first k tiles of that engine's range are done".
        s_ws_dve = ent(nc.semaphore("s_ws_dve"))
        s_ws_act = ent(nc.semaphore("s_ws_act"))
        s_mm = ent(nc.semaphore("s_mm"))   # psum groups done (+1 each)
        s_ev = ent(nc.semaphore("s_ev"))   # evict+bias adds done (+1 each)
        # Per-slot DMA-completion sems: DMA completions across a shared sem
        # are unordered, so "sem >= 16*(k+1)" would not mean "DMA k landed".
        # One sem per ring slot with at most one DMA in flight per slot makes
        # the thresholds sound.
        s_xd = [ent(nc.semaphore(f"s_xd{i}")) for i in range(NB_X)]
        s_wd = [ent(nc.semaphore(f"s_wd{i}")) for i in range(NB_W)]
        s_od = [ent(nc.semaphore(f"s_od{i}")) for i in range(NB_O)]
        all_sems = [s_bd, s_bs, s_xs, s_ws_dve, s_ws_act, s_mm, s_ev,
                    *s_xd, *s_wd, *s_od]

        N_DVE_BLK = min(2, NT)        # W blocks signed by DVE
        N_DVE_SIGNS = N_DVE_BLK * KT  # tiles 0..N_DVE_SIGNS-1 belong to DVE

        def wslice(n):
            return slice(n * nfree, (n + 1) * nfree)

        def wait_wsign(eng, j):
            """Wait until W tile j (global index) has been sign-converted."""
            if j < N_DVE_SIGNS:
                eng.wait_ge(s_ws_dve, j + 1)
            else:
                eng.wait_ge(s_ws_act, j - N_DVE_SIGNS + 1)

        U8 = mybir.dt.uint8

        def dve_sign(vector, dst_ap, src_ap):
            """+-1 fp8e4 sign via bit ops: (hi_byte(bf16) & 0x80) | 0x38."""
            return vector.tensor_scalar(
                out=dst_ap.bitcast(U8),
                in0=src_ap.bitcast(U8)[:, 1::2],
                scalar1=0x80,
                scalar2=0x38,
                op0=mybir.AluOpType.bitwise_and,
                op1=mybir.AluOpType.bitwise_or,
            )

        with nc.Block() as block:

            @block.sync
            def _(sync):
                sync.dma_start(out=bsb[:, :], in_=bias_b[:, :]).then_inc(s_bd, 16)
                # prologue: interleave x tiles with W block-0 tiles
                for kt in range(KT):
                    if kt >= NB_X:
                        sync.wait_ge(s_xs, kt - NB_X + 1)
                    sync.dma_start(
                        out=xsb[:, kt % NB_X, :], in_=xT[kt * P:(kt + 1) * P, :]
                    ).then_inc(s_xd[kt % NB_X], 16)
                    if kt >= NB_W:
                        wait_wsign(sync, kt - NB_W)
                    sync.dma_start(
                        out=wsb[:, kt % NB_W, :], in_=W[kt * P:(kt + 1) * P, wslice(0)]
                    ).then_inc(s_wd[kt % NB_W], 16)
                for n in range(1, NT):
                    for kt in range(KT):
                        j = n * KT + kt
                        wait_wsign(sync, j - NB_W)
                        sync.dma_start(
                            out=wsb[:, j % NB_W, :],
                            in_=W[kt * P:(kt + 1) * P, wslice(n)],
                        ).then_inc(s_wd[j % NB_W], 16)

            @block.scalar
            def _(scalar):
                # x signs first: they gate the PE prologue
                for kt in range(KT):
                    scalar.wait_ge(s_xd[kt % NB_X], 16 * (kt // NB_X + 1))
                    scalar.activation(
                        bx[:, kt, :], xsb[:, kt % NB_X, :], SIGN
                    ).then_inc(s_xs, 1)
                scalar.wait_ge(s_bd, 16)
                scalar.activation(bsb[:, :], bsb[:, :], SIGN).then_inc(s_bs, 1)
                # W signs for blocks 2+ (blocks 0..1 are DVE's)
                for n in range(N_DVE_BLK, NT):
                    for kt in range(KT):
                        j = n * KT + kt
                        if kt == 0 and n >= 2:
                            # wb slot n%2 is free once PE drained block n-2
                            scalar.wait_ge(s_mm, MT * (n - 1))
                        scalar.wait_ge(s_wd[j % NB_W], 16 * (j // NB_W + 1))
                        scalar.activation(
                            wb[:, n % 2, kt, :], wsb[:, j % NB_W, :], SIGN
                        ).then_inc(s_ws_act, 1)

            @block.tensor
            def _(tensor):
                # Block 0 runs k-major across all MT psum banks: each freshly
                # signed k-pair immediately unlocks MT matmuls, so the PE is
                # never starved behind the serial prologue sign chain.
                for kk in range(KK):
                    tensor.wait_ge(s_xs, 2 * kk + 2)
                    tensor.wait_ge(s_ws_dve, 2 * kk + 2)
                    for m in range(MT):
                        mm = tensor.matmul(
                            pst[m % 8][:, :],
                            bx[:, 2 * kk:2 * kk + 2, m * P:(m + 1) * P],
                            wb[:, 0, 2 * kk:2 * kk + 2, :],
                            start=(kk == 0),
                            stop=(kk == KK - 1),
                            perf_mode=mybir.MatmulPerfMode.DoubleRow,
                        )
                        if kk == KK - 1:
                            mm.then_inc(s_mm, 1)
                # Blocks 1+: m-major, one bank per group; the first m-tile of
                # each block is k-gated so a lagging sign stream degrades
                # smoothly instead of stalling the whole block.
                for n in range(1, NT):
                    for m in range(MT):
                        g = n * MT + m
                        if g >= 8:
                            tensor.wait_ge(s_ev, g - 7)
                        for kk in range(KK):
                            if m == 0:
                                wait_wsign(tensor, n * KT + 2 * kk + 1)
                            mm = tensor.matmul(
                                pst[g % 8][:, :],
                                bx[:, 2 * kk:2 * kk + 2, m * P:(m + 1) * P],
                                wb[:, n % 2, 2 * kk:2 * kk + 2, :],
                                start=(kk == 0),
                                stop=(kk == KK - 1),
                                perf_mode=mybir.MatmulPerfMode.DoubleRow,
                            )
                        mm.then_inc(s_mm, 1)

            @block.vector
            def _(vector):
                # W signs for blocks 0..1 via the u8 bit trick (ACT is busy
                # with x signs during the prologue; DVE is otherwise idle)
                for n in range(N_DVE_BLK):
                    for kt in range(KT):
                        j = n * KT + kt
                        vector.wait_ge(s_wd[j % NB_W], 16 * (j // NB_W + 1))
                        dve_sign(
                            vector, wb[:, n % 2, kt, :], wsb[:, j % NB_W, :]
                        ).then_inc(s_ws_dve, 1)
                vector.wait_ge(s_bs, 1)
                for g in range(NGRP):
                    n = g // MT
                    vector.wait_ge(s_mm, g + 1)
                    if g >= NB_O:
                        vector.wait_ge(s_od[g % NB_O], 16 * (g // NB_O))
                    vector.tensor_add(
                        osb[:, g % NB_O, :], pst[g % 8][:, :], bsb[:, wslice(n)]
                    ).then_inc(s_ev, 1)

            @block.gpsimd
            def _(gpsimd):
                for g in range(NGRP):
                    n, m = g // MT, g % MT
                    gpsimd.wait_ge(s_ev, g + 1)
                    gpsimd.dma_start(
                        out=out[m * P:(m + 1) * P, wslice(n)],
                        in_=osb[:, g % NB_O, :],
                    ).then_inc(s_od[g % NB_O], 16)
                # drain own DMAs before the end-of-block barrier
                for i in range(NB_O):
                    n_dmas = len(range(i, NGRP, NB_O))
                    gpsimd.wait_ge(s_od[i], 16 * n_dmas)

        # Block exit emitted drain + all-engine barrier: every stream is done.
        # Zero the semaphores so a re-execution of the loaded NEFF starts clean.
        for s in all_sems:
            nc.sync.sem_clear(s)

    return nc


def _prep_inputs(x, W, bias):
    """Host-side shard/layout prep: transpose x, cast to bf16 (sign-exact),
    replicate bias across the 128 partitions."""
    xT = np.ascontiguousarray(np.asarray(x).astype(ml_dtypes.bfloat16).T)
    Wb = np.ascontiguousarray(np.asarray(W).astype(ml_dtypes.bfloat16))
    bias_b = np.ascontiguousarray(
        np.broadcast_to(np.asarray(bias).astype(np.float32)[None, :], (P, D))
    )
    in_maps = []
    for c in range(N_CORES):
        in_maps.append(
            {
                "xT": np.ascontiguousarray(xT[:, c * B_SHARD:(c + 1) * B_SHARD]),
                "W": Wb,
                "bias_b": bias_b,
            }
        )
    return in_maps


def kernel(x, W, bias):
    global LAST_RESULTS
    in_maps = _prep_inputs(x, W, bias)
    nc = build_nc()
    res = run_bass_kernel_spmd(
        nc,
        in_maps,
        core_ids=list(range(N_CORES)),
        trace=bool(int(os.environ.get("KBASS_TRACE", "0"))),
    )
    LAST_RESULTS = res
    out = np.concatenate([r["out"] for r in res.results], axis=0)
    return np.ascontiguousarray(out.astype(np.float32))


# revision 13
# speedup vs baseline: 1.0672x; 1.0027x over previous
"""Trainium2 Bass kernel for nn_BINLayer (binarized dense layer).

Computes out = sign(x) @ sign(W) + sign(bias) with sign(v >= 0) = +1 else -1
(forward value of the straight-through-estimator reference).

Strategy:
  - Data-parallel shard x over batch rows: 8 cores x 1024 rows each.
    W and bias are replicated; each core computes its full [1024, 4096]
    output slice, results are concatenated on the host.
  - The host ships x transposed ([D, B_shard]) so the contraction dim lands
    on SBUF partitions, and cast to bf16 (sign-exact: bf16 has the full f32
    exponent range, so sign(bf16(v)) == sign(v) for every input value).
  - On device: sign is computed on the Scalar engine (ACT Sign activation),
    emitting +-1 directly in fp8e4. The matmul runs on the Tensor engine in
    fp8 DoubleRow mode (2 fp8 weights per PE cell, contraction 256/matmul)
    with fp32 PSUM accumulation. Since all operands are exactly +-1 and row
    sums are integers <= 4097, the result is bit-exact vs float64.
  - Bias (sign-converted on device) is added during PSUM->SBUF eviction on
    the Vector engine, fused with the copy.
"""

import os
from contextlib import ExitStack

import numpy as np
import ml_dtypes

import concourse.bass as bass
from concourse import mybir
from concourse.bass_utils import run_bass_kernel_spmd

P = 128
D = 4096
B = 8192
N_CORES = 8
B_SHARD = B // N_CORES  # 1024
NFREE = 512  # psum free dim (one bank of fp32)

F32 = mybir.dt.float32
BF16 = mybir.dt.bfloat16
FP8 = mybir.dt.float8e4

SIGN = mybir.ActivationFunctionType.Sign

# Stash of the most recent BassKernelResults (exec_time_ns etc) for test.py.
LAST_RESULTS = None


def build_nc(d=D, b_shard=B_SHARD, nfree=NFREE, use_fp8=True):
    """Build the per-core Bass program (raw bass: explicit engine streams and
    semaphores — the toolchain here rejects DMAs carrying >1 sync wait, so all
    waits are sequencer instructions). Every core runs this same program on
    its own batch shard.

    Engine assignment:
      SP (sync)    input DMAs (bias, xT tiles, W tiles)
      ACT (scalar) all sign conversions (bf16 -> +-1 in fp8)
      PE (tensor)  matmuls (fp8 DoubleRow, fp32 PSUM accumulate)
      DVE (vector) PSUM->SBUF eviction fused with bias add
      POOL(gpsimd) output DMAs
    """
    KT = d // P        # contraction tiles of 128
    MT = b_shard // P  # output row tiles of 128
    NT = d // nfree    # output col blocks of nfree
    KK = KT // 2       # DoubleRow pairs
    NB_X = min(4, KT)  # x staging ring slots
    NB_W = min(8, KT)  # W staging ring slots
    NB_O = 8           # out staging ring slots
    NGRP = NT * MT     # psum accumulation groups
    cdt = FP8 if use_fp8 else BF16

    nc = bass.Bass()
    xT = nc.declare_dram_parameter("xT", [d, b_shard], BF16, isOutput=False)
    W = nc.declare_dram_parameter("W", [d, d], BF16, isOutput=False)
    bias_b = nc.declare_dram_parameter("bias_b", [P, d], F32, isOutput=False)
    out = nc.declare_dram_parameter("out", [b_shard, d], F32, isOutput=True)

    with ExitStack() as ctx:
        ent = ctx.enter_context
        xsb = ent(nc.sbuf_tensor("xsb", [P, NB_X, b_shard], BF16))
        bx = ent(nc.sbuf_tensor("bx", [P, KT, b_shard], cdt))
        wsb = ent(nc.sbuf_tensor("wsb", [P, NB_W, nfree], BF16))
        wb = ent(nc.sbuf_tensor("wb", [P, 2, KT, nfree], cdt))
        bsb = ent(nc.sbuf_tensor("bsb", [P, d], F32))
        osb = ent(nc.sbuf_tensor("osb", [P, NB_O, nfree], F32))
        pst = [ent(nc.psum_tensor(f"pst{b}", [P, nfree], F32)) for b in range(8)]

        s_bd = ent(nc.semaphore("s_bd"))   # bias dma done (+16)
        s_bs = ent(nc.semaphore("s_bs"))   # bias signed (+1)
        # The prologue sign work (x + W block 0) gates everything, so it is
        # spread over three otherwise-idle engines, each with its own sem so
        # every count stays single-producer ("sem >= k" == "that engine's
        # first k tiles are done"):
        #   ACT: x even tiles, bias, W blocks 2+   (Sign activation)
        #   DVE: x odd tiles, W block 1, evictions (u8 bit trick)
        #   POOL: W block 0, out DMAs              (u8 bit trick)
        s_xs_a = ent(nc.semaphore("s_xs_a"))  # x even signs (ACT)
        s_xs_d = ent(nc.semaphore("s_xs_d"))  # x odd signs (DVE)
        s_wsd = ent(nc.semaphore("s_wsd"))    # W block-0/1 signs (DVE)
        s_wsa = ent(nc.semaphore("s_wsa"))    # W block-2+ signs (ACT)
        s_mm = ent(nc.semaphore("s_mm"))   # psum groups done (+1 each)
        s_ev = ent(nc.semaphore("s_ev"))   # evict+bias adds done (+1 each)
        # Per-slot DMA-completion sems: DMA completions across a shared sem
        # are unordered, so "sem >= 16*(k+1)" would not mean "DMA k landed".
        # One sem per ring slot with at most one DMA in flight per slot makes
        # the thresholds sound.
        s_xd = [ent(nc.semaphore(f"s_xd{i}")) for i in range(NB_X)]
        s_wd = [ent(nc.semaphore(f"s_wd{i}")) for i in range(NB_W)]
        s_od = [ent(nc.semaphore(f"s_od{i}")) for i in range(NB_O)]
        all_sems = [s_bd, s_bs, s_xs_a, s_xs_d, s_wsd, s_wsa, s_mm,
                    s_ev, *s_xd, *s_wd, *s_od]

        def wslice(n):
            return slice(n * nfree, (n + 1) * nfree)

        N_DVE_BLK = min(2, NT)        # W blocks signed by DVE
        N_DVE_SIGNS = N_DVE_BLK * KT

        def wait_wsign(eng, j):
            """Wait until W tile j (global index) has been sign-converted."""
            if j < N_DVE_SIGNS:
                eng.wait_ge(s_wsd, j + 1)
            else:
                eng.wait_ge(s_wsa, j - N_DVE_SIGNS + 1)

        def wait_xsign(eng, kt):
            """Wait until x tile kt has been sign-converted."""
            if kt % 2 == 0:
                eng.wait_ge(s_xs_a, kt // 2 + 1)
            else:
                eng.wait_ge(s_xs_d, kt // 2 + 1)

        U8 = mybir.dt.uint8

        def dve_sign(vector, dst_ap, src_ap):
            """+-1 fp8e4 sign via bit ops: (hi_byte(bf16) & 0x80) | 0x38."""
            return vector.tensor_scalar(
                out=dst_ap.bitcast(U8),
                in0=src_ap.bitcast(U8)[:, 1::2],
                scalar1=0x80,
                scalar2=0x38,
                op0=mybir.AluOpType.bitwise_and,
                op1=mybir.AluOpType.bitwise_or,
            )

        with nc.Block() as block:

            @block.sync
            def _(sync):
                # prologue: interleave x tiles with W block-0 tiles; the bias
                # DMA is issued after them (it isn't needed until the first
                # eviction, and at the head of the ring it would delay every
                # prologue tile by its ~5us transfer)
                for kt in range(KT):
                    if kt >= NB_X:
                        wait_xsign(sync, kt - NB_X)
                    sync.dma_start(
                        out=xsb[:, kt % NB_X, :], in_=xT[kt * P:(kt + 1) * P, :]
                    ).then_inc(s_xd[kt % NB_X], 16)
                    if kt >= NB_W:
                        wait_wsign(sync, kt - NB_W)
                    sync.dma_start(
                        out=wsb[:, kt % NB_W, :], in_=W[kt * P:(kt + 1) * P, wslice(0)]
                    ).then_inc(s_wd[kt % NB_W], 16)
                sync.dma_start(out=bsb[:, :], in_=bias_b[:, :]).then_inc(s_bd, 16)
                for n in range(1, NT):
                    for kt in range(KT):
                        j = n * KT + kt
                        wait_wsign(sync, j - NB_W)
                        sync.dma_start(
                            out=wsb[:, j % NB_W, :],
                            in_=W[kt * P:(kt + 1) * P, wslice(n)],
                        ).then_inc(s_wd[j % NB_W], 16)

            @block.scalar
            def _(scalar):
                # x even-tile signs (odd tiles are DVE's): they gate the PE
                # prologue
                for kt in range(0, KT, 2):
                    scalar.wait_ge(s_xd[kt % NB_X], 16 * (kt // NB_X + 1))
                    scalar.activation(
                        bx[:, kt, :], xsb[:, kt % NB_X, :], SIGN
                    ).then_inc(s_xs_a, 1)
                scalar.wait_ge(s_bd, 16)
                scalar.activation(bsb[:, :], bsb[:, :], SIGN).then_inc(s_bs, 1)
                # W signs for blocks 2+ (blocks 0..1 are DVE's)
                for n in range(N_DVE_BLK, NT):
                    for kt in range(KT):
                        j = n * KT + kt
                        if kt == 0:
                            # wb slot n%2 is free once PE drained block n-2
                            scalar.wait_ge(s_mm, MT * (n - 1))
                        scalar.wait_ge(s_wd[j % NB_W], 16 * (j // NB_W + 1))
                        scalar.activation(
                            wb[:, n % 2, kt, :], wsb[:, j % NB_W, :], SIGN
                        ).then_inc(s_wsa, 1)

            @block.tensor
            def _(tensor):
                # Block 0 runs k-major across all MT psum banks: each freshly
                # signed k-pair immediately unlocks MT matmuls, so the PE is
                # never starved behind the serial prologue sign chain.
                for kk in range(KK):
                    wait_xsign(tensor, 2 * kk)
                    wait_xsign(tensor, 2 * kk + 1)
                    tensor.wait_ge(s_wsd, 2 * kk + 2)
                    for m in range(MT):
                        mm = tensor.matmul(
                            pst[m % 8][:, :],
                            bx[:, 2 * kk:2 * kk + 2, m * P:(m + 1) * P],
                            wb[:, 0, 2 * kk:2 * kk + 2, :],
                            start=(kk == 0),
                            stop=(kk == KK - 1),
                            perf_mode=mybir.MatmulPerfMode.DoubleRow,
                        )
                        if kk == KK - 1:
                            mm.then_inc(s_mm, 1)
                # Blocks 1+: m-major, one bank per group; the first m-tile of
                # each block is k-gated so a lagging sign stream degrades
                # smoothly instead of stalling the whole block.
                for n in range(1, NT):
                    for m in range(MT):
                        g = n * MT + m
                        if g >= 8:
                            tensor.wait_ge(s_ev, g - 7)
                        for kk in range(KK):
                            if m == 0:
                                wait_wsign(tensor, n * KT + 2 * kk + 1)
                            mm = tensor.matmul(
                                pst[g % 8][:, :],
                                bx[:, 2 * kk:2 * kk + 2, m * P:(m + 1) * P],
                                wb[:, n % 2, 2 * kk:2 * kk + 2, :],
                                start=(kk == 0),
                                stop=(kk == KK - 1),
                                perf_mode=mybir.MatmulPerfMode.DoubleRow,
                            )
                        mm.then_inc(s_mm, 1)

            @block.vector
            def _(vector):
                # Block-0 prologue: x odd-tile signs interleaved with W
                # block-0 signs in exactly PE consumption order (u8 bit trick)
                for kk in range(KK):
                    kt = 2 * kk + 1
                    vector.wait_ge(s_xd[kt % NB_X], 16 * (kt // NB_X + 1))
                    dve_sign(
                        vector, bx[:, kt, :], xsb[:, kt % NB_X, :]
                    ).then_inc(s_xs_d, 1)
                    for j in (2 * kk, 2 * kk + 1):
                        vector.wait_ge(s_wd[j % NB_W], 16 * (j // NB_W + 1))
                        dve_sign(
                            vector, wb[:, 0, j, :], wsb[:, j % NB_W, :]
                        ).then_inc(s_wsd, 1)
                # W block-1 signs
                for kt in range(KT if NT > 1 else 0):
                    j = KT + kt
                    vector.wait_ge(s_wd[j % NB_W], 16 * (j // NB_W + 1))
                    dve_sign(
                        vector, wb[:, 1, kt, :], wsb[:, j % NB_W, :]
                    ).then_inc(s_wsd, 1)
                vector.wait_ge(s_bs, 1)
                for g in range(NGRP):
                    n = g // MT
                    vector.wait_ge(s_mm, g + 1)
                    if g >= NB_O:
                        vector.wait_ge(s_od[g % NB_O], 16 * (g // NB_O))
                    vector.tensor_add(
                        osb[:, g % NB_O, :], pst[g % 8][:, :], bsb[:, wslice(n)]
                    ).then_inc(s_ev, 1)

            @block.gpsimd
            def _(gpsimd):
                for g in range(NGRP):
                    n, m = g // MT, g % MT
                    gpsimd.wait_ge(s_ev, g + 1)
                    gpsimd.dma_start(
                        out=out[m * P:(m + 1) * P, wslice(n)],
                        in_=osb[:, g % NB_O, :],
                    ).then_inc(s_od[g % NB_O], 16)
                # drain own DMAs before the end-of-block barrier
                for i in range(NB_O):
                    n_dmas = len(range(i, NGRP, NB_O))
                    gpsimd.wait_ge(s_od[i], 16 * n_dmas)

        # Block exit emitted drain + all-engine barrier: every stream is done.
        # Zero the semaphores so a re-execution of the loaded NEFF starts clean.
        for s in all_sems:
            nc.sync.sem_clear(s)

    return nc


def _prep_inputs(x, W, bias):
    """Host-side shard/layout prep: transpose x, cast to bf16 (sign-exact),
    replicate bias across the 128 partitions."""
    xT = np.ascontiguousarray(np.asarray(x).astype(ml_dtypes.bfloat16).T)
    Wb = np.ascontiguousarray(np.asarray(W).astype(ml_dtypes.bfloat16))
    bias_b = np.ascontiguousarray(
        np.broadcast_to(np.asarray(bias).astype(np.float32)[None, :], (P, D))
    )
    in_maps = []
    for c in range(N_CORES):
        in_maps.append(
            {
                "xT": np.ascontiguousarray(xT[:, c * B_SHARD:(c + 1) * B_SHARD]),
                "W": Wb,
                "bias_b": bias_b,
            }
        )
    return in_maps


def kernel(x, W, bias):
    global LAST_RESULTS
    in_maps = _prep_inputs(x, W, bias)
    nc = build_nc()
    res = run_bass_kernel_spmd(
        nc,
        in_maps,
        core_ids=list(range(N_CORES)),
        trace=bool(int(os.environ.get("KBASS_TRACE", "0"))),
    )
    LAST_RESULTS = res
    out = np.concatenate([r["out"] for r in res.results], axis=0)
    return np.ascontiguousarray(out.astype(np.float32))


# revision 15
# speedup vs baseline: 1.0967x; 1.0276x over previous
"""Trainium2 Bass kernel for nn_BINLayer (binarized dense layer).

Computes out = sign(x) @ sign(W) + sign(bias) with sign(v >= 0) = +1 else -1
(forward value of the straight-through-estimator reference).

Strategy:
  - Data-parallel shard x over batch rows: 8 cores x 1024 rows each.
    W and bias are replicated; each core computes its full [1024, 4096]
    output slice, results are concatenated on the host.
  - The host ships x transposed ([D, B_shard]) so the contraction dim lands
    on SBUF partitions, and cast to bf16 (sign-exact: bf16 has the full f32
    exponent range, so sign(bf16(v)) == sign(v) for every input value).
  - On device: sign is computed on the Scalar engine (ACT Sign activation),
    emitting +-1 directly in fp8e4. The matmul runs on the Tensor engine in
    fp8 DoubleRow mode (2 fp8 weights per PE cell, contraction 256/matmul)
    with fp32 PSUM accumulation. Since all operands are exactly +-1 and row
    sums are integers <= 4097, the result is bit-exact vs float64.
  - Bias (sign-converted on device) is added during PSUM->SBUF eviction on
    the Vector engine, fused with the copy.
"""

import os
from contextlib import ExitStack

import numpy as np
import ml_dtypes

import concourse.bass as bass
from concourse import mybir
from concourse.bass_utils import run_bass_kernel_spmd

P = 128
D = 4096
B = 8192
N_CORES = 8
B_SHARD = B // N_CORES  # 1024
NFREE = 512  # psum free dim (one bank of fp32)

F32 = mybir.dt.float32
BF16 = mybir.dt.bfloat16
FP8 = mybir.dt.float8e4

SIGN = mybir.ActivationFunctionType.Sign

# Stash of the most recent BassKernelResults (exec_time_ns etc) for test.py.
LAST_RESULTS = None


def build_nc(d=D, b_shard=B_SHARD, nfree=NFREE, use_fp8=True):
    """Build the per-core Bass program (raw bass: explicit engine streams and
    semaphores — the toolchain here rejects DMAs carrying >1 sync wait, so all
    waits are sequencer instructions). Every core runs this same program on
    its own batch shard.

    Engine assignment:
      SP (sync)    input DMAs, batched 4 k-tiles each (the ~0.6us per-trigger
                   issue cost would otherwise throttle the prologue)
      ACT (scalar) x even-tile signs + bias sign (Sign activation)
      DVE (vector) x odd-tile signs, ALL W signs (u8 bit trick, ~3x faster
                   than ACT's Sign), PSUM->SBUF eviction fused with bias add
      PE (tensor)  matmuls (fp8 DoubleRow, fp32 PSUM accumulate)
      POOL(gpsimd) output DMAs
    """
    KT = d // P        # contraction tiles of 128
    MT = b_shard // P  # output row tiles of 128
    NT = d // nfree    # output col blocks of nfree
    KK = KT // 2       # DoubleRow pairs
    XB = min(4, KT)    # k-tiles per input DMA batch
    NXD = KT // XB     # x DMA batches
    NWD = KT // XB     # W DMA batches per block
    NB_X = min(2, NXD)       # x staging ring slots (one batch each)
    NB_W = min(4, NWD * NT)  # W staging ring slots (one batch each)
    NB_O = 8           # out staging ring slots
    NGRP = NT * MT     # psum accumulation groups
    cdt = FP8 if use_fp8 else BF16

    nc = bass.Bass()
    xT = nc.declare_dram_parameter("xT", [d, b_shard], BF16, isOutput=False)
    W = nc.declare_dram_parameter("W", [d, d], BF16, isOutput=False)
    bias_b = nc.declare_dram_parameter("bias_b", [P, d], F32, isOutput=False)
    out = nc.declare_dram_parameter("out", [b_shard, d], F32, isOutput=True)

    with ExitStack() as ctx:
        ent = ctx.enter_context
        xsb = ent(nc.sbuf_tensor("xsb", [P, NB_X, XB, b_shard], BF16))
        bx = ent(nc.sbuf_tensor("bx", [P, KT, b_shard], cdt))
        wsb = ent(nc.sbuf_tensor("wsb", [P, NB_W, XB, nfree], BF16))
        wb = ent(nc.sbuf_tensor("wb", [P, 2, KT, nfree], cdt))
        bsb = ent(nc.sbuf_tensor("bsb", [P, d], F32))
        osb = ent(nc.sbuf_tensor("osb", [P, NB_O, nfree], F32))
        pst = [ent(nc.psum_tensor(f"pst{b}", [P, nfree], F32)) for b in range(8)]

        s_bd = ent(nc.semaphore("s_bd"))   # bias dma done (+16)
        s_bs = ent(nc.semaphore("s_bs"))   # bias signed (+1)
        # Sign sems are single-producer so "sem >= k" == "that engine's first
        # k tiles are done" (engine instruction streams complete in order).
        s_xs_a = ent(nc.semaphore("s_xs_a"))  # x even signs (ACT)
        s_xs_d = ent(nc.semaphore("s_xs_d"))  # x odd signs (DVE)
        s_wsd = ent(nc.semaphore("s_wsd"))    # W signs, all blocks (DVE)
        s_mm = ent(nc.semaphore("s_mm"))   # psum groups done (+1 each)
        s_ev = ent(nc.semaphore("s_ev"))   # evict+bias adds done (+1 each)
        # Per-slot DMA-completion sems: DMA completions across a shared sem
        # are unordered, so "sem >= 16*(k+1)" would not mean "DMA k landed".
        # One sem per ring slot with at most one DMA in flight per slot makes
        # the thresholds sound.
        s_xd = [ent(nc.semaphore(f"s_xd{i}")) for i in range(NB_X)]
        s_wd = [ent(nc.semaphore(f"s_wd{i}")) for i in range(NB_W)]
        s_od = [ent(nc.semaphore(f"s_od{i}")) for i in range(NB_O)]
        all_sems = [s_bd, s_bs, s_xs_a, s_xs_d, s_wsd, s_mm, s_ev,
                    *s_xd, *s_wd, *s_od]

        def wslice(n):
            return slice(n * nfree, (n + 1) * nfree)

        def wait_xsign(eng, kt):
            """Wait until x tile kt has been sign-converted."""
            if kt % 2 == 0:
                eng.wait_ge(s_xs_a, kt // 2 + 1)
            else:
                eng.wait_ge(s_xs_d, kt // 2 + 1)

        U8 = mybir.dt.uint8

        def dve_sign(vector, dst_ap, src_ap):
            """+-1 fp8e4 sign via bit ops: (hi_byte(bf16) & 0x80) | 0x38."""
            return vector.tensor_scalar(
                out=dst_ap.bitcast(U8),
                in0=src_ap.bitcast(U8)[:, 1::2],
                scalar1=0x80,
                scalar2=0x38,
                op0=mybir.AluOpType.bitwise_and,
                op1=mybir.AluOpType.bitwise_or,
            )

        def batched(dram_slice):
            """[XB*P, C] DRAM slice -> [P, XB, C] AP (row s*P + p -> [p, s])."""
            return dram_slice.rearrange("(s p) c -> p s c", p=P)

        with nc.Block() as block:

            @block.sync
            def _(sync):
                # prologue: alternate x batches with W block-0 batches; the
                # bias DMA is issued after them (it isn't needed until the
                # first eviction, and at the head of the ring it would delay
                # every prologue tile by its ~5us transfer)
                for i in range(NXD):
                    if i >= NB_X:
                        # slot free once both parities of batch i-NB_X signed
                        base = (i - NB_X) * XB
                        wait_xsign(sync, base + XB - 1)
                        wait_xsign(sync, base + XB - 2)
                    sync.dma_start(
                        out=xsb[:, i % NB_X],
                        in_=batched(xT[i * XB * P:(i + 1) * XB * P, :]),
                    ).then_inc(s_xd[i % NB_X], 16)
                    if i >= NB_W:
                        sync.wait_ge(s_wsd, (i - NB_W) * XB + XB)
                    sync.dma_start(
                        out=wsb[:, i % NB_W],
                        in_=batched(W[i * XB * P:(i + 1) * XB * P, wslice(0)]),
                    ).then_inc(s_wd[i % NB_W], 16)
                sync.dma_start(out=bsb[:, :], in_=bias_b[:, :]).then_inc(s_bd, 16)
                for n in range(1, NT):
                    for i in range(NWD):
                        bi = n * NWD + i
                        if bi >= NB_W:
                            sync.wait_ge(s_wsd, (bi - NB_W) * XB + XB)
                        sync.dma_start(
                            out=wsb[:, bi % NB_W],
                            in_=batched(W[i * XB * P:(i + 1) * XB * P, wslice(n)]),
                        ).then_inc(s_wd[bi % NB_W], 16)

            @block.scalar
            def _(scalar):
                # x even-tile signs (odd tiles are DVE's): they gate the PE
                # prologue
                for kt in range(0, KT, 2):
                    bi = kt // XB
                    scalar.wait_ge(s_xd[bi % NB_X], 16 * (bi // NB_X + 1))
                    scalar.activation(
                        bx[:, kt, :], xsb[:, bi % NB_X, kt % XB, :], SIGN
                    ).then_inc(s_xs_a, 1)
                scalar.wait_ge(s_bd, 16)
                scalar.activation(bsb[:, :], bsb[:, :], SIGN).then_inc(s_bs, 1)

            @block.tensor
            def _(tensor):
                # Block 0 runs k-major across all MT psum banks: each freshly
                # signed k-pair immediately unlocks MT matmuls, so the PE is
                # never starved behind the serial prologue sign chain.
                for kk in range(KK):
                    wait_xsign(tensor, 2 * kk)
                    wait_xsign(tensor, 2 * kk + 1)
                    tensor.wait_ge(s_wsd, 2 * kk + 2)
                    for m in range(MT):
                        mm = tensor.matmul(
                            pst[m % 8][:, :],
                            bx[:, 2 * kk:2 * kk + 2, m * P:(m + 1) * P],
                            wb[:, 0, 2 * kk:2 * kk + 2, :],
                            start=(kk == 0),
                            stop=(kk == KK - 1),
                            perf_mode=mybir.MatmulPerfMode.DoubleRow,
                        )
                        if kk == KK - 1:
                            mm.then_inc(s_mm, 1)
                # Blocks 1+: m-major, one bank per group; the first m-tile of
                # each block is k-gated so a lagging sign stream degrades
                # smoothly instead of stalling the whole block.
                for n in range(1, NT):
                    for m in range(MT):
                        g = n * MT + m
                        if g >= 8:
                            tensor.wait_ge(s_ev, g - 7)
                        for kk in range(KK):
                            if m == 0:
                                tensor.wait_ge(s_wsd, n * KT + 2 * kk + 2)
                            mm = tensor.matmul(
                                pst[g % 8][:, :],
                                bx[:, 2 * kk:2 * kk + 2, m * P:(m + 1) * P],
                                wb[:, n % 2, 2 * kk:2 * kk + 2, :],
                                start=(kk == 0),
                                stop=(kk == KK - 1),
                                perf_mode=mybir.MatmulPerfMode.DoubleRow,
                            )
                        mm.then_inc(s_mm, 1)

            @block.vector
            def _(vector):
                def wsign(j, slot):
                    """Sign W tile j (global index) into wb slot."""
                    bi = j // XB
                    vector.wait_ge(s_wd[bi % NB_W], 16 * (bi // NB_W + 1))
                    dve_sign(
                        vector,
                        wb[:, slot, (j % KT), :],
                        wsb[:, bi % NB_W, j % XB, :],
                    ).then_inc(s_wsd, 1)

                # Block-0 prologue: x odd-tile signs interleaved with W
                # block-0 signs in exactly PE consumption order
                for kk in range(KK):
                    kt = 2 * kk + 1
                    bi = kt // XB
                    vector.wait_ge(s_xd[bi % NB_X], 16 * (bi // NB_X + 1))
                    dve_sign(
                        vector, bx[:, kt, :], xsb[:, bi % NB_X, kt % XB, :]
                    ).then_inc(s_xs_d, 1)
                    wsign(2 * kk, 0)
                    wsign(2 * kk + 1, 0)
                # W block-1 signs
                for kt in range(KT if NT > 1 else 0):
                    wsign(KT + kt, 1)
                vector.wait_ge(s_bs, 1)
                # steady state: trail block n's evictions, then (while PE
                # works block n+1) sign W block n+2
                for n in range(NT):
                    for m in range(MT):
                        g = n * MT + m
                        vector.wait_ge(s_mm, g + 1)
                        if g >= NB_O:
                            vector.wait_ge(s_od[g % NB_O], 16 * (g // NB_O))
                        vector.tensor_add(
                            osb[:, g % NB_O, :], pst[g % 8][:, :],
                            bsb[:, wslice(n)],
                        ).then_inc(s_ev, 1)
                    if n + 2 < NT:
                        # wb slot n%2 was freed by block n's last group, which
                        # the eviction loop above already waited on
                        for kt in range(KT):
                            wsign((n + 2) * KT + kt, n % 2)

            @block.gpsimd
            def _(gpsimd):
                for g in range(NGRP):
                    n, m = g // MT, g % MT
                    gpsimd.wait_ge(s_ev, g + 1)
                    gpsimd.dma_start(
                        out=out[m * P:(m + 1) * P, wslice(n)],
                        in_=osb[:, g % NB_O, :],
                    ).then_inc(s_od[g % NB_O], 16)
                # drain own DMAs before the end-of-block barrier
                for i in range(NB_O):
                    n_dmas = len(range(i, NGRP, NB_O))
                    gpsimd.wait_ge(s_od[i], 16 * n_dmas)

        # Block exit emitted drain + all-engine barrier: every stream is done.
        # Zero the semaphores so a re-execution of the loaded NEFF starts clean.
        for s in all_sems:
            nc.sync.sem_clear(s)

    return nc


def _prep_inputs(x, W, bias):
    """Host-side shard/layout prep: transpose x, cast to bf16 (sign-exact),
    replicate bias across the 128 partitions."""
    xT = np.ascontiguousarray(np.asarray(x).astype(ml_dtypes.bfloat16).T)
    Wb = np.ascontiguousarray(np.asarray(W).astype(ml_dtypes.bfloat16))
    bias_b = np.ascontiguousarray(
        np.broadcast_to(np.asarray(bias).astype(np.float32)[None, :], (P, D))
    )
    in_maps = []
    for c in range(N_CORES):
        in_maps.append(
            {
                "xT": np.ascontiguousarray(xT[:, c * B_SHARD:(c + 1) * B_SHARD]),
                "W": Wb,
                "bias_b": bias_b,
            }
        )
    return in_maps


def kernel(x, W, bias):
    global LAST_RESULTS
    in_maps = _prep_inputs(x, W, bias)
    nc = build_nc()
    res = run_bass_kernel_spmd(
        nc,
        in_maps,
        core_ids=list(range(N_CORES)),
        trace=bool(int(os.environ.get("KBASS_TRACE", "0"))),
    )
    LAST_RESULTS = res
    out = np.concatenate([r["out"] for r in res.results], axis=0)
    return np.ascontiguousarray(out.astype(np.float32))
